# revision 17
# baseline (speedup 1.0000x reference)
"""Trainium2 Bass kernel for nn_MultiDense: y[b,n,o] = sum_i x[b,n,i]*A[0,n,o,i] + Bp[0,n,o].

Sharding: tensor-parallel over the nsplit group axis — 256 groups / 8 cores
= 32 independent (2048x256) @ (256x256)^T GEMMs per core.

Per core, per group n:
  lhsT = x_n^T  (i on partitions, batch on free)   <- host pre-transposed
  rhs  = A_n^T  (i on partitions, out on free)     <- host pre-transposed
  psum[b_tile, o] accumulated over 2 k-tiles; 4 batch-tiles share one
  2-bank PSUM super-tile so the bias add + evacuation is a single VectorE
  op (broadcast bias) and the store is one contiguous 512KB DMA.

The kernel is HBM-bandwidth bound, so x/A/y move as fp16 (fp32 PSUM
accumulation; measured rel err ~3.6e-4 on HW). x/at are i-major so each
SBUF partition's DRAM slice is contiguous; y is stored as (n, s, p, j, o)
blocks so stores are contiguous too. The host folds y back to (b, n, o)
and upcasts to fp32.
"""

import sys
import functools

sys.path.insert(0, "/opt/trn_rl_repo")

import numpy as np

B_SZ, NSPLIT, OUT, IN = 2048, 256, 256, 256
NCORES = 8
GPC = NSPLIT // NCORES  # 32 groups per core
P = 128
KT = IN // P  # 2 k-tiles
SB = 8  # batch tiles per PSUM super-tile (4 PSUM banks)
ST = B_SZ // (P * SB)  # 2 super-tiles per group
GL = 2  # groups loaded per input DMA (bigger contiguous chunks)


@functools.lru_cache(maxsize=1)
def _build():
    from concourse import bacc, mybir, tile

    F32 = mybir.dt.float32
    F16 = mybir.dt.float16

    nc = bacc.Bacc("TRN2", target_bir_lowering=False, debug=False)
    # All layouts keep each SBUF partition's DRAM slice contiguous:
    # x loads 16KB/partition, at 2KB, bias 4KB, y stores 8KB.
    M = GPC // GL
    xt = nc.dram_tensor("xt", [M, P, GL, KT, B_SZ], F16, kind="ExternalInput")
    at = nc.dram_tensor("at", [M, P, GL, KT, OUT], F16, kind="ExternalInput")
    bias = nc.dram_tensor("bias", [M, P, GL, OUT], F16, kind="ExternalInput")
    y = nc.dram_tensor("y", [GPC, P, ST, SB, OUT], F16, kind="ExternalOutput")

    with tile.TileContext(nc) as tc:
        with (
            tc.tile_pool(name="xp", bufs=4) as xp,
            tc.tile_pool(name="ap", bufs=4) as ap_,
            tc.tile_pool(name="bp", bufs=4) as bp,
            tc.tile_pool(name="op", bufs=6) as op,
            tc.tile_pool(name="ps", bufs=2, space="PSUM") as ps,
        ):
            for m in range(GPC // GL):
                x_t = xp.tile([P, GL, KT, B_SZ], F16, tag="x")
                nc.sync.dma_start(x_t[:], xt[m])
                a_t = ap_.tile([P, GL, KT, OUT], F16, tag="a")
                nc.sync.dma_start(a_t[:], at[m])
                b_t = bp.tile([P, GL, OUT], F16, tag="b")
                nc.sync.dma_start(b_t[:], bias[m])

                for g in range(GL):
                    n = m * GL + g
                    b_bc = b_t[:, g, :].rearrange("p (u o) -> p u o", u=1).to_broadcast(
                        (P, SB, OUT)
                    )
                    o_t = op.tile([P, ST, SB, OUT], F16, tag="o")
                    for s in range(ST):
                        p = ps.tile([P, SB, OUT], F32, tag="p")
                        for j in range(SB):
                            bsl = slice((s * SB + j) * P, (s * SB + j + 1) * P)
                            for k in range(KT):
                                nc.tensor.matmul(
                                    p[:, j, :],
                                    x_t[:, g, k, bsl],
                                    a_t[:, g, k, :],
                                    start=(k == 0),
                                    stop=(k == KT - 1),
                                )
                        nc.vector.tensor_add(o_t[:, s], p[:], b_bc)
                    nc.scalar.dma_start(y[n], o_t[:])

    nc.finalize()
    return nc


def _shard_inputs(x, A, Bp):
    """Slice + relayout the full inputs into per-core in_maps."""
    M = GPC // GL
    in_maps = []
    for c in range(NCORES):
        ng = slice(c * GPC, (c + 1) * GPC)
        # x[:, n, i] -> xt[m, i_lo, g, k, b], fp16
        xs = np.ascontiguousarray(
            x[:, ng, :]
            .transpose(1, 2, 0)
            .reshape(M, GL, KT, P, B_SZ)
            .transpose(0, 3, 1, 2, 4)
            .astype(np.float16)
        )
        # A[0, n, o, i] -> at[m, i_lo, g, k, o], fp16
        ats = np.ascontiguousarray(
            A[0, ng]
            .reshape(M, GL, OUT, KT, P)
            .transpose(0, 4, 1, 3, 2)
            .astype(np.float16)
        )
        # bias[m, p, g, o] = Bp[0, n, o], replicated across the 128 partitions
        bs = np.ascontiguousarray(
            np.broadcast_to(
                Bp[0, ng].reshape(M, GL, OUT)[:, None, :, :], (M, P, GL, OUT)
            ).astype(np.float16)
        )
        in_maps.append({"xt": xs, "at": ats, "bias": bs})
    return in_maps


def _run(in_maps, **kwargs):
    from concourse.bass_utils import run_bass_kernel_spmd

    nc = _build()
    return run_bass_kernel_spmd(nc, in_maps, list(range(NCORES)), **kwargs)


def kernel(x, A, Bp):
    x = np.ascontiguousarray(x, dtype=np.float32)
    A = np.ascontiguousarray(A, dtype=np.float32)
    Bp = np.ascontiguousarray(Bp, dtype=np.float32)
    res = _run(_shard_inputs(x, A, Bp))
    # per-core y is (GPC, P, ST, SB, OUT) fp16 with b = s*(P*SB) + j*P + p;
    # stack cores on the group axis, then fold back to (B, NSPLIT, OUT) fp32.
    yg = np.concatenate([r["y"] for r in res.results], axis=0)
    return (
        np.ascontiguousarray(yg.transpose(2, 3, 1, 0, 4))
        .reshape(B_SZ, NSPLIT, OUT)
        .astype(np.float32)
    )


# revision 18
# speedup vs baseline: 1.0566x; 1.0566x over previous
"""Trainium2 Bass kernel for nn_MultiDense: y[b,n,o] = sum_i x[b,n,i]*A[0,n,o,i] + Bp[0,n,o].

Sharding: tensor-parallel over the nsplit group axis — 256 groups / 8 cores
= 32 independent (2048x256) @ (256x256)^T GEMMs per core.

Per core, per group n:
  lhsT = x_n^T  (i on partitions, batch on free)   <- host pre-transposed
  rhs  = A_n^T  (i on partitions, out on free)     <- host pre-transposed
  psum[b_tile, o] accumulated over 2 k-tiles; 8 batch-tiles share one
  4-bank PSUM super-tile so the bias add + evacuation is a single VectorE
  op per super-tile (broadcast bias) and the store is one contiguous
  2MB DMA per group (issued on the otherwise-idle ScalarE queue).

The kernel is HBM-bandwidth bound (~70MB/core at ~400GB/s), so x/A/bias/y
move as fp16 with fp32 PSUM accumulation; measured rel err 3.6e-4 on HW
(fp32r variant measured 1.46e-4 at ~445us if tighter accuracy is ever
needed). All DRAM layouts keep each SBUF partition's slice contiguous
(x 16KB, y 8KB per partition) so DMA packets stay large. The host folds
y back to (b, n, o) and upcasts to fp32.
"""

import sys
import functools

sys.path.insert(0, "/opt/trn_rl_repo")

import numpy as np

B_SZ, NSPLIT, OUT, IN = 2048, 256, 256, 256
NCORES = 8
GPC = NSPLIT // NCORES  # 32 groups per core
P = 128
KT = IN // P  # 2 k-tiles
SB = 8  # batch tiles per PSUM super-tile (4 PSUM banks)
ST = B_SZ // (P * SB)  # 2 super-tiles per group
GL = 2  # groups loaded per input DMA (bigger contiguous chunks)


@functools.lru_cache(maxsize=1)
def _build():
    from concourse import bacc, mybir, tile

    F32 = mybir.dt.float32
    F16 = mybir.dt.float16

    nc = bacc.Bacc("TRN2", target_bir_lowering=False, debug=False)
    # All layouts keep each SBUF partition's DRAM slice contiguous:
    # x loads 16KB/partition, at 2KB, bias 4KB, y stores 8KB.
    M = GPC // GL
    xt = nc.dram_tensor("xt", [M, P, GL, KT, B_SZ], F16, kind="ExternalInput")
    at = nc.dram_tensor("at", [M, P, GL, KT, OUT], F16, kind="ExternalInput")
    bias = nc.dram_tensor("bias", [M, P, GL, OUT], F16, kind="ExternalInput")
    y = nc.dram_tensor("y", [GPC, P, ST, SB, OUT], F16, kind="ExternalOutput")

    with tile.TileContext(nc) as tc:
        with (
            tc.tile_pool(name="xp", bufs=4) as xp,
            tc.tile_pool(name="ap", bufs=4) as ap_,
            tc.tile_pool(name="bp", bufs=4) as bp,
            tc.tile_pool(name="op", bufs=6) as op,
            tc.tile_pool(name="ps", bufs=2, space="PSUM") as ps,
        ):
            for m in range(GPC // GL):
                x_t = xp.tile([P, GL, KT, B_SZ], F16, tag="x")
                nc.sync.dma_start(x_t[:], xt[m])
                a_t = ap_.tile([P, GL, KT, OUT], F16, tag="a")
                nc.sync.dma_start(a_t[:], at[m])
                b_t = bp.tile([P, GL, OUT], F16, tag="b")
                nc.sync.dma_start(b_t[:], bias[m])

                for g in range(GL):
                    n = m * GL + g
                    b_bc = b_t[:, g, :].rearrange("p (u o) -> p u o", u=1).to_broadcast(
                        (P, SB, OUT)
                    )
                    o_t = op.tile([P, ST, SB, OUT], F16, tag="o")
                    for s in range(ST):
                        p = ps.tile([P, SB, OUT], F32, tag="p")
                        for j in range(SB):
                            bsl = slice((s * SB + j) * P, (s * SB + j + 1) * P)
                            for k in range(KT):
                                nc.tensor.matmul(
                                    p[:, j, :],
                                    x_t[:, g, k, bsl],
                                    a_t[:, g, k, :],
                                    start=(k == 0),
                                    stop=(k == KT - 1),
                                )
                        nc.vector.tensor_add(o_t[:, s], p[:], b_bc)
                    nc.scalar.dma_start(y[n], o_t[:])

    nc.finalize()
    return nc


def _shard_inputs(x, A, Bp):
    """Slice + relayout the full inputs into per-core in_maps."""
    M = GPC // GL
    in_maps = []
    for c in range(NCORES):
        ng = slice(c * GPC, (c + 1) * GPC)
        # x[:, n, i] -> xt[m, i_lo, g, k, b], fp16
        xs = np.ascontiguousarray(
            x[:, ng, :]
            .transpose(1, 2, 0)
            .reshape(M, GL, KT, P, B_SZ)
            .transpose(0, 3, 1, 2, 4)
            .astype(np.float16)
        )
        # A[0, n, o, i] -> at[m, i_lo, g, k, o], fp16
        ats = np.ascontiguousarray(
            A[0, ng]
            .reshape(M, GL, OUT, KT, P)
            .transpose(0, 4, 1, 3, 2)
            .astype(np.float16)
        )
        # bias[m, p, g, o] = Bp[0, n, o], replicated across the 128 partitions
        bs = np.ascontiguousarray(
            np.broadcast_to(
                Bp[0, ng].reshape(M, GL, OUT)[:, None, :, :], (M, P, GL, OUT)
            ).astype(np.float16)
        )
        in_maps.append({"xt": xs, "at": ats, "bias": bs})
    return in_maps


def _run(in_maps, **kwargs):
    from concourse.bass_utils import run_bass_kernel_spmd

    nc = _build()
    return run_bass_kernel_spmd(nc, in_maps, list(range(NCORES)), **kwargs)


def kernel(x, A, Bp):
    x = np.ascontiguousarray(x, dtype=np.float32)
    A = np.ascontiguousarray(A, dtype=np.float32)
    Bp = np.ascontiguousarray(Bp, dtype=np.float32)
    res = _run(_shard_inputs(x, A, Bp))
    # per-core y is (GPC, P, ST, SB, OUT) fp16 with b = s*(P*SB) + j*P + p;
    # stack cores on the group axis, then fold back to (B, NSPLIT, OUT) fp32.
    yg = np.concatenate([r["y"] for r in res.results], axis=0)
    return (
        np.ascontiguousarray(yg.transpose(2, 3, 1, 0, 4))
        .reshape(B_SZ, NSPLIT, OUT)
        .astype(np.float32)
    )
